# revision 5
# baseline (speedup 1.0000x reference)
"""Causal transformer layer (B=4,T=2048,D=1024,F=4096,H=16) on 8 trn2 NeuronCores.

L1 (attention): core = (batch b, head-group g of 8 heads).
L2 (MLP):       core = contiguous 1024-token chunk of the flattened (B,T).

Matmul operands are bf16 (1 cyc/row on PE vs 4 for fp32); PSUM accumulation
stays fp32. Residuals and LayerNorm run in fp32. Output error is further
damped by the 0.01 gamma scales on both branches.

L1 attention computes scores pre-transposed (out = k_tile^T-major
[keys, queries]), so exp() output feeds the AV matmul directly as the
moving operand with V stationary -- no per-tile transposes of the
probability matrix. V carries an extra ones column so the softmax
denominator falls out of the same PSUM accumulation (row 64 of the AV
output).
"""

import sys

sys.path.insert(0, "/opt/trn_rl_repo")

import numpy as np
import ml_dtypes

import concourse.bass as bass
import concourse.tile as tile
from concourse import bacc, mybir
from concourse.bass_utils import run_bass_kernel_spmd
from concourse.masks import make_identity

F32 = mybir.dt.float32
BF16 = mybir.dt.bfloat16
NPBF = ml_dtypes.bfloat16

B, T, D, F, H, HD = 4, 2048, 1024, 4096, 16, 64
EPS = 1e-6
NT = T // 128          # 16 token tiles (L1)
KD = D // 128          # 8 k-tiles over D
SCALE = HD ** -0.5     # 0.125, folded into exp()
EXPF = mybir.ActivationFunctionType.Exp


def _bcast(ap, p=128):
    """Broadcast a [N] DRAM vector across p partitions -> [p, N] DMA source."""
    return bass.AP(tensor=ap.tensor, offset=ap.offset, ap=[[0, p], *list(ap.ap)])


# --------------------------------------------------------------------------
# L1: attention partial.  Inputs (per core):
#   x        [2048, 1024]  token-major batch slice (f32)
#   wqkv     [1024, 1536]  host-packed cols: [q(512) | k(512) | v(512)] (bf16)
#   wo       [512, 1024]   rows of w_o for this head group (bf16)
#   cos_rep  [128, 2048]   cos[t, :32].T replicated to 4 row-blocks of 32 (bf16)
#   sin_srep [128, 2048]   sin.T with signs [-,+,-,+] per 32-row block (bf16)
#   ln1s, ln1b, gattn [1024] (f32)
#   triu     [128, 128]    upper-triangular ones incl diagonal (bf16)
# Output: y [2048, 1024] = (attn_partial @ wo) * gamma_attn  (token-major, bf16)
# --------------------------------------------------------------------------
def build_l1():
    nc = bacc.Bacc("TRN2", target_bir_lowering=False, debug=False, num_devices=8)
    x = nc.dram_tensor("x", [T, D], F32, kind="ExternalInput").ap()
    wqkv = nc.dram_tensor("wqkv", [D, 1536], BF16, kind="ExternalInput").ap()
    wo = nc.dram_tensor("wo", [512, D], BF16, kind="ExternalInput").ap()
    cos_rep = nc.dram_tensor("cos_rep", [128, T], BF16, kind="ExternalInput").ap()
    sin_srep = nc.dram_tensor("sin_srep", [128, T], BF16, kind="ExternalInput").ap()
    ln1s = nc.dram_tensor("ln1s", [D], F32, kind="ExternalInput").ap()
    ln1b = nc.dram_tensor("ln1b", [D], F32, kind="ExternalInput").ap()
    gattn = nc.dram_tensor("gattn", [D], F32, kind="ExternalInput").ap()
    triu = nc.dram_tensor("triu", [128, 128], BF16, kind="ExternalInput").ap()
    y = nc.dram_tensor("y", [T, D], BF16, kind="ExternalOutput").ap()

    wqkv_r = wqkv.rearrange("(k p) n -> p k n", p=128)   # [128, 8, 1536]
    wo_r = wo.rearrange("(k p) n -> p k n", p=128)       # [128, 4, 1024]

    with nc.allow_low_precision("bf16 kernel; rel tolerance 2e-2"), \
         tile.TileContext(nc) as tc:
        with (
            tc.tile_pool(name="singles", bufs=1) as singles,
            tc.tile_pool(name="xnT", bufs=1) as xnTp,
            tc.tile_pool(name="vall", bufs=1) as vallp,
            tc.tile_pool(name="attT", bufs=1) as attTp,
        ):
            ident = singles.tile([128, 128], BF16)
            make_identity(nc, ident)
            eps_t = singles.tile([128, 1], F32)
            nc.vector.memset(eps_t, EPS)
            triu_t = singles.tile([128, 128], BF16)
            nc.sync.dma_start(out=triu_t, in_=triu)
            cos_t = singles.tile([128, T], BF16)
            nc.sync.dma_start(out=cos_t, in_=cos_rep)
            sin_t = singles.tile([128, T], BF16)
            nc.sync.dma_start(out=sin_t, in_=sin_srep)
            ln1s_b = singles.tile([128, D], F32)
            nc.sync.dma_start(out=ln1s_b, in_=_bcast(ln1s))
            ln1b_b = singles.tile([128, D], F32)
            nc.sync.dma_start(out=ln1b_b, in_=_bcast(ln1b))
            gattn_b = singles.tile([128, D], F32)
            nc.sync.dma_start(out=gattn_b, in_=_bcast(gattn))
            ones64 = singles.tile([1, 64], BF16)
            nc.vector.memset(ones64, 1.0)

            xnT = [xnTp.tile([128, T], BF16, tag=f"xnT{k}", name=f"xnT{k}")
                   for k in range(KD)]
            v_g = [vallp.tile([128, NT, 65], BF16, tag=f"v{g}", name=f"v{g}")
                   for g in range(8)]
            for g in range(8):
                nc.vector.memset(v_g[g][:, :, 64:65], 1.0)
            attT = [attTp.tile([128, T], BF16, tag=f"attT{p}", name=f"attT{p}")
                    for p in range(4)]

            # ---- Phase A: LN1 + transpose to feature-major (bf16) ----
            with (
                tc.tile_pool(name="lnw", bufs=3) as lnw,
                tc.tile_pool(name="lnst", bufs=4) as lnst,
                tc.tile_pool(name="ptr", bufs=4, space="PSUM") as ptr,
            ):
                for tt in range(NT):
                    x_t = lnw.tile([128, D], F32, tag="x_t")
                    nc.sync.dma_start(out=x_t, in_=x[tt * 128:(tt + 1) * 128, :])
                    st = lnst.tile([128, 2, 6], F32, tag="st")
                    for s in range(2):
                        nc.vector.bn_stats(out=st[:, s, :], in_=x_t[:, s * 512:(s + 1) * 512])
                    mv = lnst.tile([128, 2], F32, tag="mv")
                    nc.vector.bn_aggr(out=mv, in_=st)
                    rstd = lnst.tile([128, 1], F32, tag="rstd")
                    nc.scalar.activation(out=rstd, in_=mv[:, 1:2],
                                         func=mybir.ActivationFunctionType.Sqrt,
                                         bias=eps_t)
                    nc.vector.reciprocal(out=rstd, in_=rstd)
                    xn_t = lnw.tile([128, D], F32, tag="xn_t")
                    nc.vector.tensor_scalar(out=xn_t, in0=x_t,
                                            scalar1=mv[:, 0:1], scalar2=rstd,
                                            op0=mybir.AluOpType.subtract,
                                            op1=mybir.AluOpType.mult)
                    nc.vector.tensor_mul(out=xn_t, in0=xn_t, in1=ln1s_b)
                    xn_b = lnw.tile([128, D], BF16, tag="xn_b")
                    nc.vector.tensor_add(out=xn_b, in0=xn_t, in1=ln1b_b)
                    for k in range(KD):
                        pt = ptr.tile([128, 128], BF16, tag="pt")
                        nc.tensor.transpose(pt, xn_b[:, k * 128:(k + 1) * 128], ident)
                        if k % 2 == 0:
                            nc.vector.tensor_copy(out=xnT[k][:, tt * 128:(tt + 1) * 128], in_=pt)
                        else:
                            nc.scalar.copy(out=xnT[k][:, tt * 128:(tt + 1) * 128], in_=pt)

            # ---- Phase A2: V projection for all 8 heads + ones column ----
            with (
                tc.tile_pool(name="wv", bufs=1) as wvp,
                tc.tile_pool(name="pv", bufs=2, space="PSUM") as pvp,
            ):
                wv_all = wvp.tile([128, KD, 512], BF16)
                nc.sync.dma_start(out=wv_all, in_=wqkv_r[:, :, 1024:1536])
                for tt in range(NT):
                    pv = pvp.tile([128, 512], F32, tag="pv")
                    for k in range(KD):
                        nc.tensor.matmul(pv, xnT[k][:, tt * 128:(tt + 1) * 128],
                                         wv_all[:, k, :],
                                         start=(k == 0), stop=(k == KD - 1))
                    for g in range(8):
                        dst = v_g[g][:, tt, 0:64]
                        src = pv[:, g * 64:(g + 1) * 64]
                        if g % 2 == 0:
                            nc.vector.tensor_copy(out=dst, in_=src)
                        else:
                            nc.scalar.copy(out=dst, in_=src)

            # ---- Phase B: per head-pair Q/K projection + rope + attention ----
            for hp in range(4):
                with (
                    tc.tile_pool(name=f"w{hp}", bufs=1) as wp,
                    tc.tile_pool(name=f"qk{hp}", bufs=1) as qkp,
                    tc.tile_pool(name=f"rot{hp}", bufs=2) as rotp,
                ):
                    wq = wp.tile([128, KD, 128], BF16, tag="wq")
                    nc.sync.dma_start(out=wq, in_=wqkv_r[:, :, hp * 128:(hp + 1) * 128])
                    wk = wp.tile([128, KD, 128], BF16, tag="wk")
                    nc.sync.dma_start(out=wk, in_=wqkv_r[:, :, 512 + hp * 128:512 + (hp + 1) * 128])
                    qfin = qkp.tile([128, T], BF16, tag="qfin", name=f"qfin{hp}")
                    kfin = qkp.tile([128, T], BF16, tag="kfin", name=f"kfin{hp}")
                    with tc.tile_pool(name=f"pqk{hp}", bufs=2, space="PSUM") as pqk:
                        for c in range(4):
                            cs = slice(c * 512, (c + 1) * 512)
                            for wt, dst in ((wq, qfin), (wk, kfin)):
                                pp = pqk.tile([128, 512], F32, tag="pp")
                                for k in range(KD):
                                    nc.tensor.matmul(pp, wt[:, k, :], xnT[k][:, cs],
                                                     start=(k == 0), stop=(k == KD - 1))
                                nc.scalar.copy(out=dst[:, cs], in_=pp)
                                rt = rotp.tile([128, 512], BF16, tag="rt")
                                for blk in range(4):
                                    src = slice(blk * 32 + (32 if blk % 2 == 0 else -32),
                                                blk * 32 + (64 if blk % 2 == 0 else 0))
                                    nc.sync.dma_start(out=rt[blk * 32:(blk + 1) * 32, :],
                                                      in_=dst[src, cs])
                                nc.vector.tensor_mul(out=rt, in0=rt, in1=sin_t[:, cs])
                                nc.vector.tensor_mul(out=dst[:, cs], in0=dst[:, cs], in1=cos_t[:, cs])
                                nc.vector.tensor_add(out=dst[:, cs], in0=dst[:, cs], in1=rt)

                    with (
                        tc.tile_pool(name=f"e{hp}", bufs=4) as ep,
                        tc.tile_pool(name=f"nrm{hp}", bufs=2) as nrmp,
                        tc.tile_pool(name=f"ps{hp}", bufs=3, space="PSUM") as psp,
                        tc.tile_pool(name=f"pav{hp}", bufs=2, space="PSUM") as pavp,
                        tc.tile_pool(name=f"pden{hp}", bufs=2, space="PSUM") as pdenp,
                    ):
                        for h in range(2):
                            g = hp * 2 + h
                            rb = 64 * h
                            for qc in range(4):
                                q_sl = slice(qc * 512, (qc + 1) * 512)
                                last = 4 * qc + 3
                                pav = pavp.tile([65, 512], F32, tag="pav")
                                pend = []
                                for kt in range(last + 1):
                                    ps = psp.tile([128, 512], F32, tag="ps")
                                    nc.tensor.matmul(ps,
                                                     kfin[rb:rb + 64, kt * 128:(kt + 1) * 128],
                                                     qfin[rb:rb + 64, q_sl],
                                                     start=True, stop=True)
                                    e = ep.tile([128, 512], BF16, tag="e")
                                    nc.scalar.activation(out=e, in_=ps, func=EXPF,
                                                         scale=SCALE)
                                    j = kt - 4 * qc
                                    if j >= 0:
                                        if j > 0:
                                            nc.vector.memset(e[:, :j * 128], 0.0)
                                        nc.vector.tensor_mul(out=e[:, j * 128:(j + 1) * 128],
                                                             in0=e[:, j * 128:(j + 1) * 128],
                                                             in1=triu_t)
                                    pend.append((kt, e))
                                    if len(pend) > 2:
                                        k0, e0 = pend.pop(0)
                                        nc.tensor.matmul(pav, v_g[g][:, k0, :], e0,
                                                         start=(k0 == 0), stop=False)
                                for k0, e0 in pend:
                                    nc.tensor.matmul(pav, v_g[g][:, k0, :], e0,
                                                     start=(k0 == 0), stop=(k0 == last))
                                # normalize: row 64 of pav is the softmax denom
                                rden = nrmp.tile([1, 512], BF16, tag="rden")
                                nc.vector.reciprocal(out=rden, in_=pav[64:65, :])
                                pden = pdenp.tile([64, 512], F32, tag="pden")
                                nc.tensor.matmul(pden, ones64, rden, start=True, stop=True)
                                rden_b = nrmp.tile([64, 512], BF16, tag="rden_b")
                                nc.scalar.copy(out=rden_b, in_=pden)
                                nc.vector.tensor_mul(out=attT[hp][rb:rb + 64, q_sl],
                                                     in0=pav[0:64, :], in1=rden_b)

            # ---- Phase C: O projection + gamma_attn ----
            with (
                tc.tile_pool(name="wop", bufs=1) as wop,
                tc.tile_pool(name="yw", bufs=3) as yw,
                tc.tile_pool(name="po", bufs=2, space="PSUM") as pop,
            ):
                wo_t = wop.tile([128, 4, D], BF16)
                nc.sync.dma_start(out=wo_t, in_=wo_r)
                for tt in range(NT):
                    y_t = yw.tile([128, D], BF16, tag="y_t")
                    for dc in range(2):
                        po = pop.tile([128, 512], F32, tag="po")
                        for hp in range(4):
                            nc.tensor.matmul(po, attT[hp][:, tt * 128:(tt + 1) * 128],
                                             wo_t[:, hp, dc * 512:(dc + 1) * 512],
                                             start=(hp == 0), stop=(hp == 3))
                        nc.vector.tensor_mul(out=y_t[:, dc * 512:(dc + 1) * 512],
                                             in0=po, in1=gattn_b[:, dc * 512:(dc + 1) * 512])
                    nc.sync.dma_start(out=y[tt * 128:(tt + 1) * 128, :], in_=y_t)

    nc.compile()
    return nc


# --------------------------------------------------------------------------
# L2: MLP.  Inputs (per core, 1024-token chunk):
#   xc [1024, 1024] f32; ya, yb [1024, 1024] bf16; x2 = xc + ya + yb
#   ln2s, ln2b, gmlp [1024] f32
#   wg, wu [1024, 4096] bf16, wd [4096, 1024] bf16
# Output: out [1024, 1024] f32 = x2 + gmlp * (gelu_tanh(xn2@wg) * (xn2@wu)) @ wd
# --------------------------------------------------------------------------
def build_l2():
    nc = bacc.Bacc("TRN2", target_bir_lowering=False, debug=False, num_devices=8)
    TC = 1024
    NTC = TC // 128  # 8
    NF = F // 128    # 32
    xc = nc.dram_tensor("xc", [TC, D], F32, kind="ExternalInput").ap()
    ya = nc.dram_tensor("ya", [TC, D], BF16, kind="ExternalInput").ap()
    yb = nc.dram_tensor("yb", [TC, D], BF16, kind="ExternalInput").ap()
    ln2s = nc.dram_tensor("ln2s", [D], F32, kind="ExternalInput").ap()
    ln2b = nc.dram_tensor("ln2b", [D], F32, kind="ExternalInput").ap()
    gmlp = nc.dram_tensor("gmlp", [D], F32, kind="ExternalInput").ap()
    wg = nc.dram_tensor("wg", [D, F], BF16, kind="ExternalInput").ap()
    wu = nc.dram_tensor("wu", [D, F], BF16, kind="ExternalInput").ap()
    wd = nc.dram_tensor("wd", [F, D], BF16, kind="ExternalInput").ap()
    out = nc.dram_tensor("out", [TC, D], F32, kind="ExternalOutput").ap()

    wg_r = wg.rearrange("(k p) n -> p k n", p=128)   # [128, 8, 4096]
    wu_r = wu.rearrange("(k p) n -> p k n", p=128)
    wd_r = wd.rearrange("(a p) n -> p a n", p=128)   # [128, 32, 1024]

    with nc.allow_low_precision("bf16 kernel; rel tolerance 2e-2"), \
         tile.TileContext(nc) as tc:
        with (
            tc.tile_pool(name="singles", bufs=1) as singles,
            tc.tile_pool(name="x2p", bufs=1) as x2p,
            tc.tile_pool(name="xnTp", bufs=1) as xnTp,
            tc.tile_pool(name="mp", bufs=1) as mp,
        ):
            ident = singles.tile([128, 128], BF16)
            make_identity(nc, ident)
            eps_t = singles.tile([128, 1], F32)
            nc.vector.memset(eps_t, EPS)
            ln2s_b = singles.tile([128, D], F32)
            nc.sync.dma_start(out=ln2s_b, in_=_bcast(ln2s))
            ln2b_b = singles.tile([128, D], F32)
            nc.sync.dma_start(out=ln2b_b, in_=_bcast(ln2b))
            gmlp_b = singles.tile([128, D], F32)
            nc.sync.dma_start(out=gmlp_b, in_=_bcast(gmlp))

            x2 = [x2p.tile([128, D], BF16, tag=f"x2{t}", name=f"x2_{t}")
                  for t in range(NTC)]
            xn2T = [xnTp.tile([128, TC], BF16, tag=f"xn2T{k}", name=f"xn2T{k}")
                    for k in range(KD)]
            m = [mp.tile([128, TC], BF16, tag=f"m{fi}", name=f"m{fi}")
                 for fi in range(NF)]

            # ---- Phase 1: residual add + LN2 + transpose ----
            with (
                tc.tile_pool(name="lnw", bufs=3) as lnw,
                tc.tile_pool(name="lnst", bufs=4) as lnst,
                tc.tile_pool(name="ptr", bufs=4, space="PSUM") as ptr,
            ):
                for tt in range(NTC):
                    rs = slice(tt * 128, (tt + 1) * 128)
                    a_t = lnw.tile([128, D], BF16, tag="a_t")
                    nc.sync.dma_start(out=a_t, in_=ya[rs, :])
                    b_t = lnw.tile([128, D], BF16, tag="b_t")
                    nc.sync.dma_start(out=b_t, in_=yb[rs, :])
                    c_t = lnw.tile([128, D], F32, tag="c_t")
                    nc.sync.dma_start(out=c_t, in_=xc[rs, :])
                    ab_t = lnw.tile([128, D], F32, tag="ab_t")
                    nc.vector.tensor_add(out=ab_t, in0=a_t, in1=b_t)
                    nc.vector.tensor_add(out=x2[tt], in0=ab_t, in1=c_t)
                    st = lnst.tile([128, 2, 6], F32, tag="st")
                    for s in range(2):
                        nc.vector.bn_stats(out=st[:, s, :], in_=x2[tt][:, s * 512:(s + 1) * 512])
                    mv = lnst.tile([128, 2], F32, tag="mv")
                    nc.vector.bn_aggr(out=mv, in_=st)
                    rstd = lnst.tile([128, 1], F32, tag="rstd")
                    nc.scalar.activation(out=rstd, in_=mv[:, 1:2],
                                         func=mybir.ActivationFunctionType.Sqrt,
                                         bias=eps_t)
                    nc.vector.reciprocal(out=rstd, in_=rstd)
                    xn_t = lnw.tile([128, D], F32, tag="xn_t")
                    nc.vector.tensor_scalar(out=xn_t, in0=x2[tt],
                                            scalar1=mv[:, 0:1], scalar2=rstd,
                                            op0=mybir.AluOpType.subtract,
                                            op1=mybir.AluOpType.mult)
                    nc.vector.tensor_mul(out=xn_t, in0=xn_t, in1=ln2s_b)
                    xn_b = lnw.tile([128, D], BF16, tag="xn_b")
                    nc.vector.tensor_add(out=xn_b, in0=xn_t, in1=ln2b_b)
                    for k in range(KD):
                        pt = ptr.tile([128, 128], BF16, tag="pt")
                        nc.tensor.transpose(pt, xn_b[:, k * 128:(k + 1) * 128], ident)
                        if k % 2 == 0:
                            nc.vector.tensor_copy(out=xn2T[k][:, tt * 128:(tt + 1) * 128], in_=pt)
                        else:
                            nc.scalar.copy(out=xn2T[k][:, tt * 128:(tt + 1) * 128], in_=pt)

            # ---- Phase 2: gate/up -> m  (feature-major, bf16) ----
            with (
                tc.tile_pool(name="wgu", bufs=4) as wgup,
                tc.tile_pool(name="pg", bufs=2, space="PSUM") as pgp,
                tc.tile_pool(name="pu", bufs=2, space="PSUM") as pup,
            ):
                for fi in range(NF):
                    fs = slice(fi * 128, (fi + 1) * 128)
                    wg_t = wgup.tile([128, KD, 128], BF16, tag="wg")
                    nc.sync.dma_start(out=wg_t, in_=wg_r[:, :, fs])
                    wu_t = wgup.tile([128, KD, 128], BF16, tag="wu")
                    nc.sync.dma_start(out=wu_t, in_=wu_r[:, :, fs])
                    for c in range(2):
                        cs = slice(c * 512, (c + 1) * 512)
                        pgt = pgp.tile([128, 512], F32, tag="pg")
                        for k in range(KD):
                            nc.tensor.matmul(pgt, wg_t[:, k, :], xn2T[k][:, cs],
                                             start=(k == 0), stop=(k == KD - 1))
                        put = pup.tile([128, 512], F32, tag="pu")
                        for k in range(KD):
                            nc.tensor.matmul(put, wu_t[:, k, :], xn2T[k][:, cs],
                                             start=(k == 0), stop=(k == KD - 1))
                        nc.scalar.activation(out=m[fi][:, cs], in_=pgt,
                                             func=mybir.ActivationFunctionType.Gelu_apprx_tanh)
                        nc.vector.tensor_mul(out=m[fi][:, cs], in0=m[fi][:, cs], in1=put)

            # ---- Phase 3: down projection, token-major out + residual ----
            with (
                tc.tile_pool(name="wdp", bufs=2) as wdp,
                tc.tile_pool(name="ow", bufs=3) as ow,
                tc.tile_pool(name="pd", bufs=2, space="PSUM") as pdp,
            ):
                for dc in range(2):
                    ds = slice(dc * 512, (dc + 1) * 512)
                    wd_h = wdp.tile([128, NF, 512], BF16, tag="wd")
                    nc.sync.dma_start(out=wd_h, in_=wd_r[:, :, ds])
                    for tt in range(NTC):
                        o_t = ow.tile([128, 512], F32, tag="o_t")
                        pdt = pdp.tile([128, 512], F32, tag="pd")
                        for fi in range(NF):
                            nc.tensor.matmul(pdt, m[fi][:, tt * 128:(tt + 1) * 128],
                                             wd_h[:, fi, :],
                                             start=(fi == 0), stop=(fi == NF - 1))
                        nc.vector.tensor_mul(out=o_t, in0=pdt, in1=gmlp_b[:, ds])
                        nc.vector.tensor_add(out=o_t, in0=o_t, in1=x2[tt][:, ds])
                        nc.sync.dma_start(out=out[tt * 128:(tt + 1) * 128, ds], in_=o_t)

    nc.compile()
    return nc


# --------------------------------------------------------------------------
# Host orchestration
# --------------------------------------------------------------------------
def prep_l1_inputs(x, cos, sin, ln1_scale, ln1_bias, w_qkv, w_o, gamma_attn):
    cosT = np.ascontiguousarray(cos.T)          # [32, 2048]
    sinT = np.ascontiguousarray(sin.T)
    cos_rep = np.tile(cosT, (4, 1)).astype(NPBF)              # [128, 2048]
    sin_srep = np.concatenate([-sinT, sinT, -sinT, sinT], 0).astype(NPBF)
    triu = np.triu(np.ones((128, 128), np.float32)).astype(NPBF)
    wq, wk, wv = w_qkv[:, :D], w_qkv[:, D:2 * D], w_qkv[:, 2 * D:]
    maps = []
    for core in range(8):
        b, g = core // 2, core % 2
        cols = slice(g * 512, (g + 1) * 512)
        wqkv_c = np.concatenate([wq[:, cols], wk[:, cols], wv[:, cols]], 1)
        maps.append({
            "x": np.ascontiguousarray(x[b]),
            "wqkv": np.ascontiguousarray(wqkv_c).astype(NPBF),
            "wo": np.ascontiguousarray(w_o[cols, :]).astype(NPBF),
            "cos_rep": cos_rep, "sin_srep": sin_srep,
            "ln1s": ln1_scale, "ln1b": ln1_bias, "gattn": gamma_attn,
            "triu": triu,
        })
    return maps


def prep_l2_inputs(x, y_cores, ln2_scale, ln2_bias, w_gate, w_up, w_down, gamma_mlp):
    wg_b = np.asarray(w_gate).astype(NPBF)
    wu_b = np.asarray(w_up).astype(NPBF)
    wd_b = np.asarray(w_down).astype(NPBF)
    maps = []
    for core in range(8):
        b, half = core // 2, core % 2
        rs = slice(half * 1024, (half + 1) * 1024)
        maps.append({
            "xc": np.ascontiguousarray(x[b][rs]),
            "ya": np.ascontiguousarray(np.asarray(y_cores[2 * b])[rs]).astype(NPBF),
            "yb": np.ascontiguousarray(np.asarray(y_cores[2 * b + 1])[rs]).astype(NPBF),
            "ln2s": ln2_scale, "ln2b": ln2_bias, "gmlp": gamma_mlp,
            "wg": wg_b, "wu": wu_b, "wd": wd_b,
        })
    return maps


_NC_CACHE = {}


def run(x, cos, sin, ln1_scale, ln1_bias, w_qkv, w_o, gamma_attn,
        ln2_scale, ln2_bias, w_gate, w_up, w_down, gamma_mlp,
        trace=False):
    f32 = lambda a: np.asarray(a, np.float32)
    x = f32(x)
    if "l1" not in _NC_CACHE:
        _NC_CACHE["l1"] = build_l1()
    if "l2" not in _NC_CACHE:
        _NC_CACHE["l2"] = build_l2()
    m1 = prep_l1_inputs(x, f32(cos), f32(sin), f32(ln1_scale), f32(ln1_bias),
                        f32(w_qkv), f32(w_o), f32(gamma_attn))
    r1 = run_bass_kernel_spmd(_NC_CACHE["l1"], m1, core_ids=list(range(8)), trace=trace)
    y_cores = [r1.results[i]["y"] for i in range(8)]
    m2 = prep_l2_inputs(x, y_cores, f32(ln2_scale), f32(ln2_bias),
                        f32(w_gate), f32(w_up), f32(w_down), f32(gamma_mlp))
    r2 = run_bass_kernel_spmd(_NC_CACHE["l2"], m2, core_ids=list(range(8)), trace=trace)
    out = np.empty((B, T, D), np.float32)
    for core in range(8):
        b, half = core // 2, core % 2
        out[b, half * 1024:(half + 1) * 1024] = r2.results[core]["out"]
    return out, (r1, r2)


def kernel(x, cos, sin, ln1_scale, ln1_bias, w_qkv, w_o, gamma_attn,
           ln2_scale, ln2_bias, w_gate, w_up, w_down, gamma_mlp):
    """Full-input / full-output entry point. Shards across 8 NeuronCores."""
    out, _ = run(x, cos, sin, ln1_scale, ln1_bias, w_qkv, w_o, gamma_attn,
                 ln2_scale, ln2_bias, w_gate, w_up, w_down, gamma_mlp)
    return out
